# revision 11
# baseline (speedup 1.0000x reference)
"""Trainium2 Bass kernel for nn_CombinedLoss_85538568667689 (FCOS varifocal loss).

Strategy
--------
The reference does an O(N*M) dense FCOS assignment (N=507904 anchors,
M=128 annotations) followed by a varifocal loss over pred [N, 2].

Key structural facts used here:
  * The in-box condition is  l <= a <= min(r, l + radius*stride), so each
    (annotation, level) pair can claim at most floor(4.5)+1 = 5 consecutive
    anchors on that level's uniform anchor grid (radius <= 4.5).
  * For target == 0 (the overwhelming majority), the loss element is
    f0(x) = 0.75 * sigmoid(x)^2 * softplus(x)  -- a pure streaming term.
  * Positive anchors only correct that:  contrib = softplus(x) - x, at the
    assigned class channel; plus the positive count for the avg factor.

So the kernel:
  1. streams pred once, summing f0(x)  (memory-bound dense pass, sharded
     over 8 cores by anchor rows),
  2. builds the <=5-wide candidate windows for this core's 16 annotations
     x 5 levels on-chip, evaluates the exact same f32 validity predicates
     as the reference against the +-4 neighboring annotations (sorted
     onsets => min-area conflicts are local), resolving assignment,
  3. gathers pred rows at the ~640 candidate positions with one indirect
     DMA and computes the sparse correction + positive count,
  4. outputs per-core [loss_numerator_partial, npos_partial]; the host
     sums the 8 pairs and divides (the "all-reduce" of two scalars).

Activations are batched per function (all Sigmoid, then all Ln) to pay
exactly two ACT table loads; softplus(x) = -ln(sigmoid(-x)).

Anchors are the deterministic grids  arange(n)*2^(i+1) + 2^i  (exact in
f32), so anchor values are synthesized on-chip instead of re-reading the
2MB anchor arrays.
"""

import os
import numpy as np

import concourse.bass as bass
import concourse.bacc as bacc
import concourse.mybir as mybir
import concourse.tile as tile

F32 = mybir.dt.float32
I32 = mybir.dt.int32
ALU = mybir.AluOpType
ACT = mybir.ActivationFunctionType
AX = mybir.AxisListType

# ---- problem constants (hardcoded per harness contract) ----
LEVEL_LENS = [262144, 131072, 65536, 32768, 16384]
N_TOT = sum(LEVEL_LENS)            # 507904
NUM_CLASSES = 2
N_CORES = 8
NSH = N_TOT // N_CORES             # 63488 rows per core (dense pass)
M_ANN = 128
MSH = M_ANN // N_CORES             # 16 annotations per core
NLVL = 5
P = MSH * NLVL                     # 80 partitions used in sparse phase
W = 8                              # candidate window width (>=5 valid + slack)
K_NBR = 4                          # neighbor annotations each side (data: max 1)
NBR = 2 * K_NBR + 1                # 9
RATE = np.float32(22050.0 / 256.0)
SIZES = np.array([[-1.0, 0.54647175],
                  [0.54647175, 0.95482662],
                  [0.95482662, 1.587662385],
                  [1.587662385, 2.35922875],
                  [2.35922875, 1000.0]], dtype=np.float32)
LEVEL_BASE = [0]
for n in LEVEL_LENS[:-1]:
    LEVEL_BASE.append(LEVEL_BASE[-1] + n)
DENSE_F = NSH * 2 // 128           # 992


NCHUNK = 2
CH = DENSE_F // NCHUNK             # 496


def _build_program():
    nc = bacc.Bacc(None, target_bir_lowering=False)
    pred_full = nc.declare_dram_parameter("pred_full", [N_TOT, 2], F32, isOutput=False)
    pred_slice = nc.declare_dram_parameter("pred_slice", [NSH, 2], F32, isOutput=False)
    aux = nc.declare_dram_parameter("aux", [P, 3 * NBR + 16], F32, isOutput=False)
    out = nc.declare_dram_parameter("out", [1, 2], F32, isOutput=True)

    with tile.TileContext(nc) as tc:
        with tc.tile_pool(name="sp", bufs=1) as sp, \
             tc.tile_pool(name="ps", bufs=1, space="PSUM") as ps:

            # pred chunks first on the sync HWDGE ring (earliest data arrival);
            # the small aux tensor rides the gpsimd SWDGE path in parallel.
            psld = pred_slice.rearrange("(p x) c -> p (x c)", p=128)
            chs = []
            for i in range(NCHUNK):
                ch = sp.tile([128, CH], F32, tag=f"d_in{i}")
                nc.sync.dma_start(out=ch[:], in_=psld[:, i * CH:(i + 1) * CH])
                chs.append(ch)
            ax = sp.tile([P, 3 * NBR + 16], F32)
            nc.gpsimd.dma_start(out=ax[:], in_=aux[:])

            l_n = ax[:, 0:NBR]
            r_n = ax[:, NBR:2 * NBR]
            cls_n = ax[:, 2 * NBR:3 * NBR]
            l_own = ax[:, K_NBR:K_NBR + 1]
            r_own = ax[:, NBR + K_NBR:NBR + K_NBR + 1]
            cls_own = ax[:, 2 * NBR + K_NBR:2 * NBR + K_NBR + 1]
            C0 = 3 * NBR
            stride = ax[:, C0 + 0:C0 + 1]
            off = ax[:, C0 + 1:C0 + 2]
            lo = ax[:, C0 + 2:C0 + 3]
            hi = ax[:, C0 + 3:C0 + 4]
            base = ax[:, C0 + 4:C0 + 5]
            nl1 = ax[:, C0 + 5:C0 + 6]
            sinv = ax[:, C0 + 6:C0 + 7]
            ws = ax[:, C0 + 8:C0 + 16]

            # ---------- candidate window [P,W] (gather-gating chain) ----------
            # A = max(l, r - hi); window start = trunc((A-off)/stride) - 1
            astart = sp.tile([P, 1], F32)
            nc.vector.tensor_scalar(astart[:], r_own, hi, l_own, ALU.subtract, ALU.max)
            jf = sp.tile([P, 1], F32)
            nc.vector.tensor_scalar(jf[:], astart[:], off, sinv, ALU.subtract, ALU.mult)
            ji = sp.tile([P, 1], I32)
            nc.vector.tensor_copy(ji[:], jf[:])
            jst = sp.tile([P, 1], F32)
            nc.vector.tensor_copy(jst[:], ji[:])
            nc.vector.tensor_scalar(jst[:], jst[:], -1.0, None, ALU.add)
            jmat = sp.tile([P, W], F32)     # jst + w, clamped for the gather
            nc.vector.tensor_scalar(jmat[:], ws, sinv, jst[:], ALU.mult, ALU.add)
            nc.vector.tensor_scalar(jmat[:], jmat[:], 0.0, nl1, ALU.max, ALU.min)
            gidx = sp.tile([P, W], F32)
            nc.vector.tensor_scalar(gidx[:], jmat[:], base, None, ALU.add)
            gi = sp.tile([P, W], I32)
            nc.vector.tensor_copy(gi[:], gidx[:])

            # ---------- gather pred rows at candidates ----------
            gt = sp.tile([P, 2 * W], F32)
            nc.gpsimd.indirect_dma_start(
                out=gt[:],
                out_offset=None,
                in_=pred_full[:, :],
                in_offset=bass.IndirectOffsetOnAxis(ap=gi[:], axis=0),
            )
            gt3 = gt[:].rearrange("p (w c) -> p w c", c=2)
            x0 = gt3[:, :, 0]
            x1 = gt3[:, :, 1]

            # non-gating sparse prep while the gather is in flight
            a0 = sp.tile([P, 1], F32)
            nc.vector.tensor_scalar(a0[:], jst[:], stride, off, ALU.mult, ALU.add)
            a = sp.tile([P, W], F32)        # candidate anchor values (exact grid)
            nc.vector.tensor_scalar(a[:], ws, a0[:], None, ALU.add)
            rad_n = sp.tile([P, NBR], F32)  # per-class radius = 4.5 - 2*cls
            nc.vector.tensor_scalar(rad_n[:], cls_n, -2.0, 4.5, ALU.mult, ALU.add)
            rc_n = sp.tile([P, NBR], F32)   # min(r', l' + radius'*stride)
            nc.vector.scalar_tensor_tensor(
                out=rc_n[:], in0=rad_n[:], scalar=stride, in1=l_n,
                op0=ALU.mult, op1=ALU.add)
            nc.vector.tensor_tensor(rc_n[:], r_n, rc_n[:], ALU.min)
            c5 = sp.tile([P, NBR], F32)     # area' < area (strictly smaller wins)
            area_own = sp.tile([P, 1], F32)
            nc.vector.tensor_tensor(area_own[:], r_own, l_own, ALU.subtract)
            nc.vector.tensor_tensor(c5[:], r_n, l_n, ALU.subtract)
            nc.vector.tensor_scalar(c5[:], c5[:], area_own[:], None, ALU.is_lt)

            d01 = sp.tile([P, W], F32)
            nc.vector.tensor_tensor(d01[:], x1, x0, ALU.subtract)
            xs = sp.tile([P, W], F32)       # pred at assigned class channel
            nc.vector.scalar_tensor_tensor(
                out=xs[:], in0=d01[:], scalar=cls_own, in1=x0,
                op0=ALU.mult, op1=ALU.add)

            # ---------- activations ----------
            # block 1: all Sigmoid (one table load)
            sigs = []
            for i in range(NCHUNK):
                s = sp.tile([128, CH], F32, tag=f"d_sig{i}")
                nc.scalar.activation(s[:], chs[i][:], ACT.Sigmoid)
                sigs.append(s)
            sgns = []
            for i in range(NCHUNK):
                s = sp.tile([128, CH], F32, tag=f"d_sgn{i}")
                nc.scalar.activation(s[:], chs[i][:], ACT.Sigmoid, scale=-1.0)
                sgns.append(s)
            sig_s = sp.tile([P, W], F32)
            nc.scalar.activation(sig_s[:], xs[:], ACT.Sigmoid)
            sgn_s = sp.tile([P, W], F32)
            nc.scalar.activation(sgn_s[:], xs[:], ACT.Sigmoid, scale=-1.0)
            # block 2: all Ln (second table load); sparse first (it gates output)
            lg_s = sp.tile([P, W], F32)
            nc.scalar.activation(lg_s[:], sgn_s[:], ACT.Ln)
            lgs = []
            for i in range(NCHUNK):
                s = sp.tile([128, CH], F32, tag=f"d_lg{i}")
                nc.scalar.activation(s[:], sgns[i][:], ACT.Ln)
                lgs.append(s)

            # dense squares can run as soon as the sigmoids land
            t2s = []
            for i in range(NCHUNK):
                t2 = sp.tile([128, CH], F32, tag=f"d_t2{i}")
                nc.vector.tensor_tensor(t2[:], sigs[i][:], sigs[i][:], ALU.mult)
                t2s.append(t2)

            # ---------- coverage matrix [P,W,NBR] (reference predicates) ----------
            a3 = a[:, :, None].to_broadcast([P, W, NBR])
            l3 = l_n[:, None, :].to_broadcast([P, W, NBR])
            r3 = r_n[:, None, :].to_broadcast([P, W, NBR])
            rc3 = rc_n[:, None, :].to_broadcast([P, W, NBR])
            c53 = c5[:, None, :].to_broadcast([P, W, NBR])

            def t3(name):
                t = sp.tile([P, W * NBR], F32, tag=name)
                return t, t[:].rearrange("p (w m) -> p w m", m=NBR)

            ls_t, ls3 = t3("b_ls")          # a - l'
            nc.vector.tensor_tensor(ls3, a3, l3, ALU.subtract)
            rs_t, rs3 = t3("b_rs")          # r' - a
            nc.vector.tensor_tensor(rs3, r3, a3, ALU.subtract)
            mx_t, mx3 = t3("b_mx")          # max(a-l', r'-a)
            nc.vector.tensor_tensor(mx3, ls3, rs3, ALU.max)
            b1_t, b13 = t3("b_b1")
            nc.vector.tensor_tensor(b13, a3, l3, ALU.is_ge)
            b2_t, b23 = t3("b_b2")
            nc.vector.tensor_tensor(b23, a3, rc3, ALU.is_le)
            nc.vector.tensor_tensor(b13, b13, b23, ALU.mult)
            b3_t, b33 = t3("b_b3")
            nc.vector.tensor_scalar(b33, mx3, lo, None, ALU.is_ge)
            b4_t, b43 = t3("b_b4")
            nc.vector.tensor_scalar(b43, mx3, hi, None, ALU.is_le)
            nc.vector.tensor_tensor(b33, b33, b43, ALU.mult)
            cov_t, cov3 = t3("b_cov")       # valid_{m'}(a) for all neighbors
            nc.vector.tensor_tensor(cov3, b13, b33, ALU.mult)
            beat_t, beat3 = t3("b_beat")    # covered by strictly smaller area'
            nc.vector.tensor_tensor(beat3, cov3, c53, ALU.mult)
            btn = sp.tile([P, W], F32)
            nc.vector.reduce_max(btn[:, :, None], beat3, axis=AX.X)

            cov_self = cov_t[:].rearrange("p (w m) -> p w m", m=NBR)[:, :, K_NBR]
            pos = sp.tile([P, W], F32)      # own-valid & not beaten
            nc.vector.tensor_scalar(btn[:], btn[:], -1.0, 1.0, ALU.mult, ALU.add)
            nc.vector.tensor_tensor(pos[:], cov_self, btn[:], ALU.mult)

            # ---------- correction: pos * (sp(x) - x - 0.75*sig(x)^2*sp(x)) ----
            # sp = -lg ;  contrib = -(lg*(1-0.75*sig^2) + x)
            s2 = sp.tile([P, W], F32)
            nc.vector.tensor_tensor(s2[:], sig_s[:], sig_s[:], ALU.mult)
            nc.vector.tensor_scalar(s2[:], s2[:], -0.75, 1.0, ALU.mult, ALU.add)
            nc.vector.tensor_tensor(s2[:], lg_s[:], s2[:], ALU.mult)
            nc.vector.tensor_tensor(s2[:], s2[:], xs[:], ALU.add)
            nc.vector.tensor_tensor(s2[:], s2[:], pos[:], ALU.mult)
            cn = sp.tile([128, 2], F32)     # [corr_raw | npos], zero-padded to 128
            nc.vector.memset(cn[:], 0.0)
            nc.vector.reduce_sum(cn[0:P, 0:1], s2[:], axis=AX.X)
            nc.vector.reduce_sum(cn[0:P, 1:2], pos[:], axis=AX.X)

            # dense multiply-with-Ln + row-sum, fused (-0.75 folded into op0)
            accs = sp.tile([128, NCHUNK], F32)
            dump = sp.tile([128, CH], F32, tag="d_dump")
            for i in range(NCHUNK):
                nc.vector.scalar_tensor_tensor(
                    out=dump[:], in0=t2s[i][:], scalar=-0.75, in1=lgs[i][:],
                    op0=ALU.mult, op1=ALU.mult, accum_out=accs[:, i:i + 1])

            # v[:,0] = dense_true - corr_raw ; v[:,1] = npos
            v = sp.tile([128, 2], F32)
            dsum = sp.tile([128, 1], F32)
            nc.vector.reduce_sum(dsum[:], accs[:], axis=AX.X)
            nc.vector.tensor_tensor(v[0:128, 0:1], dsum[:], cn[0:128, 0:1], ALU.subtract)
            nc.vector.tensor_copy(v[0:128, 1:2], cn[0:128, 1:2])

            # single PE reduction: out[1,2] = ones^T @ v
            ones = sp.tile([128, 1], F32)
            nc.vector.memset(ones[:], 1.0)
            pd = ps.tile([1, 2], F32, tag="p_d")
            nc.tensor.matmul(out=pd[:], lhsT=ones[:], rhs=v[:], start=True, stop=True)
            outsb = sp.tile([1, 2], F32)
            nc.vector.tensor_copy(outsb[:], pd[:])
            nc.gpsimd.dma_start(out=out[:], in_=outsb[:])

    nc.finalize()
    return nc


_PROG = None


def _get_program():
    global _PROG
    if _PROG is None:
        _PROG = _build_program()
    return _PROG


def _prep_in_maps(pred, annotations):
    pred = np.ascontiguousarray(pred, dtype=np.float32)
    ann = np.ascontiguousarray(annotations, dtype=np.float32)

    # level constants, shared across cores
    lvlc = np.zeros((P, 8), dtype=np.float32)
    wstr = np.zeros((P, W), dtype=np.float32)
    for lvl in range(NLVL):
        s = np.float32(2.0 ** (lvl + 1))
        sl = slice(lvl * MSH, (lvl + 1) * MSH)
        lvlc[sl, 0] = s
        lvlc[sl, 1] = np.float32(2.0 ** lvl)
        lvlc[sl, 2] = SIZES[lvl, 0] * RATE
        lvlc[sl, 3] = SIZES[lvl, 1] * RATE
        lvlc[sl, 4] = np.float32(LEVEL_BASE[lvl])
        lvlc[sl, 5] = np.float32(LEVEL_LENS[lvl] - 1)
        lvlc[sl, 6] = np.float32(1.0 / s)
        wstr[sl, :] = np.arange(W, dtype=np.float32) * s

    # sentinel-padded annotation table for neighbor windows
    SENT = np.float32(1.0e9)
    ann_pad = np.full((M_ANN + 2 * K_NBR, 3), SENT, dtype=np.float32)
    ann_pad[:, 2] = 0.0
    ann_pad[K_NBR:K_NBR + M_ANN] = ann

    in_maps = []
    for k in range(N_CORES):
        nbr = np.zeros((MSH, 3, NBR), dtype=np.float32)
        for i in range(MSH):
            m = k * MSH + i
            nbr[i] = ann_pad[m:m + NBR].T
        ann_nbr = np.tile(nbr.reshape(MSH, 3 * NBR), (NLVL, 1))  # [80, 27]
        aux = np.concatenate([ann_nbr, lvlc, wstr], axis=1)      # [80, 43]
        in_maps.append({
            "pred_full": pred,
            "pred_slice": np.ascontiguousarray(pred[k * NSH:(k + 1) * NSH]),
            "aux": np.ascontiguousarray(aux),
        })
    return in_maps


def _finalize(outs):
    num = np.sum([o[0, 0] for o in outs], dtype=np.float64)
    npos = np.sum([o[0, 1] for o in outs], dtype=np.float64)
    return np.float32(num / max(npos, 1.0))


def kernel(pred, annotations, anchors0=None, anchors1=None, anchors2=None,
           anchors3=None, anchors4=None, **_ignored):
    nc = _get_program()
    in_maps = _prep_in_maps(np.asarray(pred), np.asarray(annotations))

    if os.environ.get("KERNEL_SIM") == "1":
        from concourse import bass_interp
        outs = []
        for k in range(N_CORES):
            sim = bass_interp.CoreSim(nc)
            for name, val in in_maps[k].items():
                sim.tensor(name)[:] = val
            sim.simulate()
            outs.append(np.array(sim.tensor("out")))
        return _finalize(outs)

    from concourse import bass_utils
    res = bass_utils.run_bass_kernel_spmd(nc, in_maps, core_ids=list(range(N_CORES)))
    return _finalize([r["out"] for r in res.results])


# revision 20
# speedup vs baseline: 1.0404x; 1.0404x over previous
"""Trainium2 Bass kernel for nn_CombinedLoss_85538568667689 (FCOS varifocal loss).

Strategy
--------
The reference does an O(N*M) dense FCOS assignment (N=507904 anchors,
M=128 annotations) followed by a varifocal loss over pred [N, 2].

Key structural facts used here:
  * The in-box condition is  l <= a <= min(r, l + radius*stride), so each
    (annotation, level) pair can claim at most floor(4.5)+1 = 5 consecutive
    anchors on that level's uniform anchor grid (radius <= 4.5).
  * For target == 0 (the overwhelming majority), the loss element is
    f0(x) = 0.75 * sigmoid(x)^2 * softplus(x)  -- a pure streaming term.
  * Positive anchors only correct that:  contrib = softplus(x) - x, at the
    assigned class channel; plus the positive count for the avg factor.

So the kernel:
  1. streams pred once, summing f0(x)  (memory-bound dense pass, sharded
     over 8 cores by anchor rows),
  2. builds the <=5-wide candidate windows for this core's 16 annotations
     x 5 levels on-chip, evaluates the exact same f32 validity predicates
     as the reference against the +-4 neighboring annotations (sorted
     onsets => min-area conflicts are local), resolving assignment,
  3. gathers pred rows at the ~640 candidate positions with one indirect
     DMA and computes the sparse correction + positive count,
  4. outputs per-core [loss_numerator_partial, npos_partial]; the host
     sums the 8 pairs and divides (the "all-reduce" of two scalars).

Activations are batched per function (all Sigmoid, then all Ln) to pay
exactly two ACT table loads; softplus(x) = -ln(sigmoid(-x)).

Anchors are the deterministic grids  arange(n)*2^(i+1) + 2^i  (exact in
f32), so anchor values are synthesized on-chip instead of re-reading the
2MB anchor arrays.
"""

import os
import numpy as np

import concourse.bass as bass
import concourse.bacc as bacc
import concourse.mybir as mybir
import concourse.tile as tile

F32 = mybir.dt.float32
I32 = mybir.dt.int32
ALU = mybir.AluOpType
ACT = mybir.ActivationFunctionType
AX = mybir.AxisListType

# ---- problem constants (hardcoded per harness contract) ----
LEVEL_LENS = [262144, 131072, 65536, 32768, 16384]
N_TOT = sum(LEVEL_LENS)            # 507904
NUM_CLASSES = 2
N_CORES = 8
NSH = N_TOT // N_CORES             # 63488 rows per core (dense pass)
M_ANN = 128
MSH = M_ANN // N_CORES             # 16 annotations per core
NLVL = 5
P = MSH * NLVL                     # 80 partitions used in sparse phase
W = 8                              # candidate window width (>=5 valid + slack)
K_NBR = 4                          # neighbor annotations each side (data: max 1)
NBR = 2 * K_NBR + 1                # 9
RATE = np.float32(22050.0 / 256.0)
SIZES = np.array([[-1.0, 0.54647175],
                  [0.54647175, 0.95482662],
                  [0.95482662, 1.587662385],
                  [1.587662385, 2.35922875],
                  [2.35922875, 1000.0]], dtype=np.float32)
LEVEL_BASE = [0]
for n in LEVEL_LENS[:-1]:
    LEVEL_BASE.append(LEVEL_BASE[-1] + n)
DENSE_F = NSH * 2 // 128           # 992


NCHUNK = 2
CH = DENSE_F // NCHUNK             # 496


def _build_program():
    nc = bacc.Bacc(None, target_bir_lowering=False)
    pred_full = nc.declare_dram_parameter("pred_full", [N_TOT, 2], F32, isOutput=False)
    pred_slice = nc.declare_dram_parameter("pred_slice", [NSH, 2], F32, isOutput=False)
    aux = nc.declare_dram_parameter("aux", [P, 3 * NBR + 16], F32, isOutput=False)
    out = nc.declare_dram_parameter("out", [1, 2], F32, isOutput=True)

    with tile.TileContext(nc) as tc:
        with tc.tile_pool(name="sp", bufs=1) as sp, \
             tc.tile_pool(name="ps", bufs=1, space="PSUM") as ps:

            # aux first (tiny, gates the sparse chain), then pred chunks --
            # all on the sync HWDGE ring.
            ax = sp.tile([P, 3 * NBR + 16], F32)
            nc.sync.dma_start(out=ax[:], in_=aux[:])
            psld = pred_slice.rearrange("(p x) c -> p (x c)", p=128)
            chs = []
            for i in range(NCHUNK):
                ch = sp.tile([128, CH], F32, tag=f"d_in{i}")
                nc.sync.dma_start(out=ch[:], in_=psld[:, i * CH:(i + 1) * CH])
                chs.append(ch)

            l_n = ax[:, 0:NBR]
            r_n = ax[:, NBR:2 * NBR]
            cls_n = ax[:, 2 * NBR:3 * NBR]
            l_own = ax[:, K_NBR:K_NBR + 1]
            r_own = ax[:, NBR + K_NBR:NBR + K_NBR + 1]
            cls_own = ax[:, 2 * NBR + K_NBR:2 * NBR + K_NBR + 1]
            C0 = 3 * NBR
            stride = ax[:, C0 + 0:C0 + 1]
            off = ax[:, C0 + 1:C0 + 2]
            lo = ax[:, C0 + 2:C0 + 3]
            hi = ax[:, C0 + 3:C0 + 4]
            base = ax[:, C0 + 4:C0 + 5]
            gmax = ax[:, C0 + 5:C0 + 6]    # base + level_len - 1
            sinv = ax[:, C0 + 6:C0 + 7]
            ws = ax[:, C0 + 8:C0 + 16]     # (w-1)*stride

            # ---------- candidate window [P,W] (gather-gating chain) ----------
            # A = max(l, r - hi); window start = trunc((A-off)/stride) - 1.
            # The -1 slack is baked into wstr = (w-1)*stride host-side.
            astart = sp.tile([P, 1], F32)
            nc.vector.tensor_scalar(astart[:], r_own, hi, l_own, ALU.subtract, ALU.max)
            jf = sp.tile([P, 1], F32)
            nc.vector.tensor_scalar(jf[:], astart[:], off, sinv, ALU.subtract, ALU.mult)
            ji = sp.tile([P, 1], I32)
            nc.vector.tensor_copy(ji[:], jf[:])
            jst = sp.tile([P, 1], F32)
            nc.vector.tensor_copy(jst[:], ji[:])
            jmat = sp.tile([P, W], F32)     # jst + (w-1)
            nc.vector.tensor_scalar(jmat[:], ws, sinv, jst[:], ALU.mult, ALU.add)
            g1 = sp.tile([P, W], F32)       # level base + clamp-low
            nc.vector.tensor_scalar(g1[:], jmat[:], base, base, ALU.add, ALU.max)
            gi = sp.tile([P, W], I32)       # clamp-high + int cast on output
            nc.vector.tensor_scalar(gi[:], g1[:], gmax, None, ALU.min)

            # ---------- gather pred rows at candidates ----------
            gt = sp.tile([P, 2 * W], F32)
            nc.gpsimd.indirect_dma_start(
                out=gt[:],
                out_offset=None,
                in_=pred_full[:, :],
                in_offset=bass.IndirectOffsetOnAxis(ap=gi[:], axis=0),
            )
            gt3 = gt[:].rearrange("p (w c) -> p w c", c=2)
            x0 = gt3[:, :, 0]
            x1 = gt3[:, :, 1]

            # non-gating sparse prep while the gather is in flight
            a0 = sp.tile([P, 1], F32)
            nc.vector.tensor_scalar(a0[:], jst[:], stride, off, ALU.mult, ALU.add)
            a = sp.tile([P, W], F32)        # candidate anchor values (exact grid)
            nc.vector.tensor_scalar(a[:], ws, a0[:], None, ALU.add)
            rad_n = sp.tile([P, NBR], F32)  # per-class radius = 4.5 - 2*cls
            nc.vector.tensor_scalar(rad_n[:], cls_n, -2.0, 4.5, ALU.mult, ALU.add)
            rc_n = sp.tile([P, NBR], F32)   # min(r', l' + radius'*stride)
            nc.vector.scalar_tensor_tensor(
                out=rc_n[:], in0=rad_n[:], scalar=stride, in1=l_n,
                op0=ALU.mult, op1=ALU.add)
            nc.vector.tensor_tensor(rc_n[:], r_n, rc_n[:], ALU.min)
            c5 = sp.tile([P, NBR], F32)     # area' < area (strictly smaller wins)
            area_own = sp.tile([P, 1], F32)
            nc.vector.tensor_tensor(area_own[:], r_own, l_own, ALU.subtract)
            nc.vector.tensor_tensor(c5[:], r_n, l_n, ALU.subtract)
            nc.vector.tensor_scalar(c5[:], c5[:], area_own[:], None, ALU.is_lt)

            d01 = sp.tile([P, W], F32)
            nc.vector.tensor_tensor(d01[:], x1, x0, ALU.subtract)
            xs = sp.tile([P, W], F32)       # pred at assigned class channel
            nc.vector.scalar_tensor_tensor(
                out=xs[:], in0=d01[:], scalar=cls_own, in1=x0,
                op0=ALU.mult, op1=ALU.add)

            # ---------- activations (Exp/Ln share ONE act table) ----------
            # sp(x) = ln(exp(x)+1);  sig(x)^2 = exp(2*(x - sp(x)))  (exact)
            es, sps, s2s_d, ts_d = [], [], [], []
            for i in range(NCHUNK):
                e = sp.tile([128, CH], F32, tag=f"d_e{i}")
                nc.scalar.activation(e[:], chs[i][:], ACT.Exp)
                es.append(e)
                spd = sp.tile([128, CH], F32, tag=f"d_sp{i}")
                nc.scalar.activation(spd[:], e[:], ACT.Ln, bias=1.0)
                sps.append(spd)
                td = sp.tile([128, CH], F32, tag=f"d_t{i}")
                nc.vector.tensor_tensor(td[:], chs[i][:], spd[:], ALU.subtract)
                ts_d.append(td)
                s2d = sp.tile([128, CH], F32, tag=f"d_s2{i}")
                nc.scalar.activation(s2d[:], td[:], ACT.Exp, scale=2.0)
                s2s_d.append(s2d)
            e_s = sp.tile([P, W], F32)
            nc.scalar.activation(e_s[:], xs[:], ACT.Exp)
            sp_s = sp.tile([P, W], F32)     # softplus at candidates
            nc.scalar.activation(sp_s[:], e_s[:], ACT.Ln, bias=1.0)
            t_s = sp.tile([P, W], F32)
            nc.vector.tensor_tensor(t_s[:], xs[:], sp_s[:], ALU.subtract)
            s2_s = sp.tile([P, W], F32)     # sigmoid^2 at candidates
            nc.scalar.activation(s2_s[:], t_s[:], ACT.Exp, scale=2.0)

            # ---------- coverage matrix [P,W,NBR] (reference predicates) ----------
            a3 = a[:, :, None].to_broadcast([P, W, NBR])
            l3 = l_n[:, None, :].to_broadcast([P, W, NBR])
            r3 = r_n[:, None, :].to_broadcast([P, W, NBR])
            rc3 = rc_n[:, None, :].to_broadcast([P, W, NBR])
            c53 = c5[:, None, :].to_broadcast([P, W, NBR])

            def t3(name):
                t = sp.tile([P, W * NBR], F32, tag=name)
                return t, t[:].rearrange("p (w m) -> p w m", m=NBR)

            # size-range sub-chain
            ls_t, ls3 = t3("b_ls")          # a - l'
            nc.vector.tensor_tensor(ls3, a3, l3, ALU.subtract)
            rs_t, rs3 = t3("b_rs")          # r' - a
            nc.vector.tensor_tensor(rs3, r3, a3, ALU.subtract)
            mx_t, mx3 = t3("b_mx")          # max(a-l', r'-a)
            nc.vector.tensor_tensor(mx3, ls3, rs3, ALU.max)
            b3_t, b33 = t3("b_b3")
            nc.vector.tensor_scalar(b33, mx3, lo, None, ALU.is_ge)
            b4_t, b43 = t3("b_b4")
            nc.vector.tensor_scalar(b43, mx3, hi, None, ALU.is_le)
            nc.vector.tensor_tensor(b33, b33, b43, ALU.mult)
            # in-box sub-chain on DVE
            b1_t, b13 = t3("b_b1")
            nc.vector.tensor_tensor(b13, a3, l3, ALU.is_ge)
            b2_t, b23 = t3("b_b2")
            nc.vector.tensor_tensor(b23, a3, rc3, ALU.is_le)
            nc.vector.tensor_tensor(b13, b13, b23, ALU.mult)
            cov_t, cov3 = t3("b_cov")       # valid_{m'}(a) for all neighbors
            nc.vector.tensor_tensor(cov3, b13, b33, ALU.mult)
            beat_t, beat3 = t3("b_beat")    # covered by strictly smaller area'
            nc.vector.tensor_tensor(beat3, cov3, c53, ALU.mult)
            btn = sp.tile([P, W], F32)
            nc.vector.reduce_max(btn[:, :, None], beat3, axis=AX.X)

            cov_self = cov_t[:].rearrange("p (w m) -> p w m", m=NBR)[:, :, K_NBR]
            pos = sp.tile([P, W], F32)      # own-valid & not beaten
            nc.vector.tensor_scalar(btn[:], btn[:], -1.0, 1.0, ALU.mult, ALU.add)
            nc.vector.tensor_tensor(pos[:], cov_self, btn[:], ALU.mult)

            # ---------- correction: pos * (sp(x) - x - 0.75*sig(x)^2*sp(x)) ----
            # contrib = sp*(1 - 0.75*sig^2) - x
            u = sp.tile([P, W], F32)
            nc.vector.tensor_scalar(u[:], s2_s[:], -0.75, 1.0, ALU.mult, ALU.add)
            nc.vector.tensor_tensor(u[:], sp_s[:], u[:], ALU.mult)
            nc.vector.tensor_tensor(u[:], u[:], xs[:], ALU.subtract)
            cn = sp.tile([128, 2], F32)     # [corr_true | npos], zero-padded to 128
            nc.vector.memset(cn[:], 0.0)
            sdump = sp.tile([P, W], F32)
            nc.vector.scalar_tensor_tensor(
                out=sdump[:], in0=u[:], scalar=0.0, in1=pos[:],
                op0=ALU.add, op1=ALU.mult, accum_out=cn[0:P, 0:1])
            nc.vector.scalar_tensor_tensor(
                out=sdump[:], in0=pos[:], scalar=0.0, in1=pos[:],
                op0=ALU.add, op1=ALU.mult, accum_out=cn[0:P, 1:2])

            # dense 0.75*sig^2*sp + row-sum, fused
            accs = sp.tile([128, NCHUNK], F32)
            dump = sp.tile([128, CH], F32, tag="d_dump")
            for i in range(NCHUNK):
                nc.vector.scalar_tensor_tensor(
                    out=dump[:], in0=s2s_d[i][:], scalar=0.75, in1=sps[i][:],
                    op0=ALU.mult, op1=ALU.mult, accum_out=accs[:, i:i + 1])

            # v[:,0] = dense_true + corr_true ; v[:,1] = npos
            v = sp.tile([128, 2], F32)
            dsum = sp.tile([128, 1], F32)
            nc.vector.reduce_sum(dsum[:], accs[:], axis=AX.X)
            nc.vector.tensor_tensor(v[0:128, 0:1], dsum[:], cn[0:128, 0:1], ALU.add)
            nc.vector.tensor_copy(v[0:128, 1:2], cn[0:128, 1:2])

            # single PE reduction: out[1,2] = ones^T @ v
            ones = sp.tile([128, 1], F32)
            nc.vector.memset(ones[:], 1.0)
            pd = ps.tile([1, 2], F32, tag="p_d")
            nc.tensor.matmul(out=pd[:], lhsT=ones[:], rhs=v[:], start=True, stop=True)
            outsb = sp.tile([1, 2], F32)
            nc.vector.tensor_copy(outsb[:], pd[:])
            nc.gpsimd.dma_start(out=out[:], in_=outsb[:])

    nc.finalize()
    return nc


_PROG = None


def _get_program():
    global _PROG
    if _PROG is None:
        _PROG = _build_program()
    return _PROG


def _prep_in_maps(pred, annotations):
    pred = np.ascontiguousarray(pred, dtype=np.float32)
    ann = np.ascontiguousarray(annotations, dtype=np.float32)

    # level constants, shared across cores
    lvlc = np.zeros((P, 8), dtype=np.float32)
    wstr = np.zeros((P, W), dtype=np.float32)
    for lvl in range(NLVL):
        s = np.float32(2.0 ** (lvl + 1))
        sl = slice(lvl * MSH, (lvl + 1) * MSH)
        lvlc[sl, 0] = s
        lvlc[sl, 1] = np.float32(2.0 ** lvl)
        lvlc[sl, 2] = SIZES[lvl, 0] * RATE
        lvlc[sl, 3] = SIZES[lvl, 1] * RATE
        lvlc[sl, 4] = np.float32(LEVEL_BASE[lvl])
        lvlc[sl, 5] = np.float32(LEVEL_BASE[lvl] + LEVEL_LENS[lvl] - 1)
        lvlc[sl, 6] = np.float32(1.0 / s)
        wstr[sl, :] = (np.arange(W, dtype=np.float32) - 1.0) * s

    # sentinel-padded annotation table for neighbor windows
    SENT = np.float32(1.0e9)
    ann_pad = np.full((M_ANN + 2 * K_NBR, 3), SENT, dtype=np.float32)
    ann_pad[:, 2] = 0.0
    ann_pad[K_NBR:K_NBR + M_ANN] = ann

    in_maps = []
    for k in range(N_CORES):
        nbr = np.zeros((MSH, 3, NBR), dtype=np.float32)
        for i in range(MSH):
            m = k * MSH + i
            nbr[i] = ann_pad[m:m + NBR].T
        ann_nbr = np.tile(nbr.reshape(MSH, 3 * NBR), (NLVL, 1))  # [80, 27]
        aux = np.concatenate([ann_nbr, lvlc, wstr], axis=1)      # [80, 43]
        in_maps.append({
            "pred_full": pred,
            "pred_slice": np.ascontiguousarray(pred[k * NSH:(k + 1) * NSH]),
            "aux": np.ascontiguousarray(aux),
        })
    return in_maps


def _finalize(outs):
    num = np.sum([o[0, 0] for o in outs], dtype=np.float64)
    npos = np.sum([o[0, 1] for o in outs], dtype=np.float64)
    return np.float32(num / max(npos, 1.0))


def kernel(pred, annotations, anchors0=None, anchors1=None, anchors2=None,
           anchors3=None, anchors4=None, **_ignored):
    nc = _get_program()
    in_maps = _prep_in_maps(np.asarray(pred), np.asarray(annotations))

    if os.environ.get("KERNEL_SIM") == "1":
        from concourse import bass_interp
        outs = []
        for k in range(N_CORES):
            sim = bass_interp.CoreSim(nc)
            for name, val in in_maps[k].items():
                sim.tensor(name)[:] = val
            sim.simulate()
            outs.append(np.array(sim.tensor("out")))
        return _finalize(outs)

    from concourse import bass_utils
    res = bass_utils.run_bass_kernel_spmd(nc, in_maps, core_ids=list(range(N_CORES)))
    return _finalize([r["out"] for r in res.results])


# revision 21
# speedup vs baseline: 1.2207x; 1.1733x over previous
"""Trainium2 Bass kernel for nn_CombinedLoss_85538568667689 (FCOS varifocal loss).

Strategy
--------
The reference does an O(N*M) dense FCOS assignment (N=507904 anchors,
M=128 annotations) followed by a varifocal loss over pred [N, 2].

Key structural facts used here:
  * The in-box condition is  l <= a <= min(r, l + radius*stride), so each
    (annotation, level) pair can claim at most floor(4.5)+1 = 5 consecutive
    anchors on that level's uniform anchor grid (radius <= 4.5).
  * For target == 0 (the overwhelming majority), the loss element is
    f0(x) = 0.75 * sigmoid(x)^2 * softplus(x)  -- a pure streaming term.
  * Positive anchors only correct that:  contrib = softplus(x) - x, at the
    assigned class channel; plus the positive count for the avg factor.

So the kernel:
  1. streams pred once, summing f0(x)  (memory-bound dense pass, sharded
     over 8 cores by anchor rows),
  2. builds the <=5-wide candidate windows for this core's 16 annotations
     x 5 levels on-chip, evaluates the exact same f32 validity predicates
     as the reference against the +-4 neighboring annotations (sorted
     onsets => min-area conflicts are local), resolving assignment,
  3. gathers pred rows at the ~640 candidate positions with one indirect
     DMA and computes the sparse correction + positive count,
  4. outputs per-core [loss_numerator_partial, npos_partial]; the host
     sums the 8 pairs and divides (the "all-reduce" of two scalars).

Activations are batched per function (all Sigmoid, then all Ln) to pay
exactly two ACT table loads; softplus(x) = -ln(sigmoid(-x)).

Anchors are the deterministic grids  arange(n)*2^(i+1) + 2^i  (exact in
f32), so anchor values are synthesized on-chip instead of re-reading the
2MB anchor arrays.
"""

import os
import numpy as np

import functools

import concourse.bass as bass
import concourse.bacc as bacc
import concourse.mybir as mybir
import concourse.tile as tile

# Both Exp and Ln live in the 'natural_log_exp_and_others' ACT table, but the
# table-load inserter may pick per-function tables, paying a ~1.3us reload on
# every Exp<->Ln switch. Strip Exp/Ln from every other set (keeping dict order,
# so act_func_set_id indices still match act_info.json) to force the shared one.
_orig_gat = bacc.get_activation_tables


@functools.cache
def _gat_one_table(arch):
    keep = "natural_log_exp_and_others"
    out = {}
    for name, funcs in _orig_gat(arch).items():
        if name != keep:
            funcs = {f for f in funcs
                     if f not in (mybir.ActivationFunctionType.Exp,
                                  mybir.ActivationFunctionType.Ln)}
        out[name] = funcs
    return out


bacc.get_activation_tables = _gat_one_table

F32 = mybir.dt.float32
I32 = mybir.dt.int32
ALU = mybir.AluOpType
ACT = mybir.ActivationFunctionType
AX = mybir.AxisListType

# ---- problem constants (hardcoded per harness contract) ----
LEVEL_LENS = [262144, 131072, 65536, 32768, 16384]
N_TOT = sum(LEVEL_LENS)            # 507904
NUM_CLASSES = 2
N_CORES = 8
NSH = N_TOT // N_CORES             # 63488 rows per core (dense pass)
M_ANN = 128
MSH = M_ANN // N_CORES             # 16 annotations per core
NLVL = 5
P = MSH * NLVL                     # 80 partitions used in sparse phase
W = 8                              # candidate window width (>=5 valid + slack)
K_NBR = 4                          # neighbor annotations each side (data: max 1)
NBR = 2 * K_NBR + 1                # 9
RATE = np.float32(22050.0 / 256.0)
SIZES = np.array([[-1.0, 0.54647175],
                  [0.54647175, 0.95482662],
                  [0.95482662, 1.587662385],
                  [1.587662385, 2.35922875],
                  [2.35922875, 1000.0]], dtype=np.float32)
LEVEL_BASE = [0]
for n in LEVEL_LENS[:-1]:
    LEVEL_BASE.append(LEVEL_BASE[-1] + n)
DENSE_F = NSH * 2 // 128           # 992


NCHUNK = 2
CH = DENSE_F // NCHUNK             # 496


def _build_program():
    nc = bacc.Bacc(None, target_bir_lowering=False)
    pred_full = nc.declare_dram_parameter("pred_full", [N_TOT, 2], F32, isOutput=False)
    pred_slice = nc.declare_dram_parameter("pred_slice", [NSH, 2], F32, isOutput=False)
    aux = nc.declare_dram_parameter("aux", [P, 3 * NBR + 16], F32, isOutput=False)
    out = nc.declare_dram_parameter("out", [1, 2], F32, isOutput=True)

    with tile.TileContext(nc) as tc:
        with tc.tile_pool(name="sp", bufs=1) as sp, \
             tc.tile_pool(name="ps", bufs=1, space="PSUM") as ps:

            # aux first (tiny, gates the sparse chain), then pred chunks --
            # all on the sync HWDGE ring.
            ax = sp.tile([P, 3 * NBR + 16], F32)
            nc.sync.dma_start(out=ax[:], in_=aux[:])
            psld = pred_slice.rearrange("(p x) c -> p (x c)", p=128)
            chs = []
            for i in range(NCHUNK):
                ch = sp.tile([128, CH], F32, tag=f"d_in{i}")
                nc.sync.dma_start(out=ch[:], in_=psld[:, i * CH:(i + 1) * CH])
                chs.append(ch)

            l_n = ax[:, 0:NBR]
            r_n = ax[:, NBR:2 * NBR]
            cls_n = ax[:, 2 * NBR:3 * NBR]
            l_own = ax[:, K_NBR:K_NBR + 1]
            r_own = ax[:, NBR + K_NBR:NBR + K_NBR + 1]
            cls_own = ax[:, 2 * NBR + K_NBR:2 * NBR + K_NBR + 1]
            C0 = 3 * NBR
            stride = ax[:, C0 + 0:C0 + 1]
            off = ax[:, C0 + 1:C0 + 2]
            lo = ax[:, C0 + 2:C0 + 3]
            hi = ax[:, C0 + 3:C0 + 4]
            base = ax[:, C0 + 4:C0 + 5]
            gmax = ax[:, C0 + 5:C0 + 6]    # base + level_len - 1
            sinv = ax[:, C0 + 6:C0 + 7]
            ws = ax[:, C0 + 8:C0 + 16]     # (w-1)*stride

            # ---------- candidate window [P,W] (gather-gating chain) ----------
            # A = max(l, r - hi); window start = trunc((A-off)/stride) - 1.
            # The -1 slack is baked into wstr = (w-1)*stride host-side.
            astart = sp.tile([P, 1], F32)
            nc.vector.tensor_scalar(astart[:], r_own, hi, l_own, ALU.subtract, ALU.max)
            jf = sp.tile([P, 1], F32)
            nc.vector.tensor_scalar(jf[:], astart[:], off, sinv, ALU.subtract, ALU.mult)
            ji = sp.tile([P, 1], I32)
            nc.vector.tensor_copy(ji[:], jf[:])
            jst = sp.tile([P, 1], F32)
            nc.vector.tensor_copy(jst[:], ji[:])
            jmat = sp.tile([P, W], F32)     # jst + (w-1)
            nc.vector.tensor_scalar(jmat[:], ws, sinv, jst[:], ALU.mult, ALU.add)
            g1 = sp.tile([P, W], F32)       # level base + clamp-low
            nc.vector.tensor_scalar(g1[:], jmat[:], base, base, ALU.add, ALU.max)
            gi = sp.tile([P, W], I32)       # clamp-high + int cast on output
            nc.vector.tensor_scalar(gi[:], g1[:], gmax, None, ALU.min)

            # ---------- gather pred rows at candidates ----------
            gt = sp.tile([P, 2 * W], F32)
            nc.gpsimd.indirect_dma_start(
                out=gt[:],
                out_offset=None,
                in_=pred_full[:, :],
                in_offset=bass.IndirectOffsetOnAxis(ap=gi[:], axis=0),
            )
            gt3 = gt[:].rearrange("p (w c) -> p w c", c=2)
            x0 = gt3[:, :, 0]
            x1 = gt3[:, :, 1]

            # non-gating sparse prep while the gather is in flight
            a0 = sp.tile([P, 1], F32)
            nc.vector.tensor_scalar(a0[:], jst[:], stride, off, ALU.mult, ALU.add)
            a = sp.tile([P, W], F32)        # candidate anchor values (exact grid)
            nc.vector.tensor_scalar(a[:], ws, a0[:], None, ALU.add)
            rad_n = sp.tile([P, NBR], F32)  # per-class radius = 4.5 - 2*cls
            nc.vector.tensor_scalar(rad_n[:], cls_n, -2.0, 4.5, ALU.mult, ALU.add)
            rc_n = sp.tile([P, NBR], F32)   # min(r', l' + radius'*stride)
            nc.vector.scalar_tensor_tensor(
                out=rc_n[:], in0=rad_n[:], scalar=stride, in1=l_n,
                op0=ALU.mult, op1=ALU.add)
            nc.vector.tensor_tensor(rc_n[:], r_n, rc_n[:], ALU.min)
            c5 = sp.tile([P, NBR], F32)     # area' < area (strictly smaller wins)
            area_own = sp.tile([P, 1], F32)
            nc.vector.tensor_tensor(area_own[:], r_own, l_own, ALU.subtract)
            nc.vector.tensor_tensor(c5[:], r_n, l_n, ALU.subtract)
            nc.vector.tensor_scalar(c5[:], c5[:], area_own[:], None, ALU.is_lt)

            d01 = sp.tile([P, W], F32)
            nc.vector.tensor_tensor(d01[:], x1, x0, ALU.subtract)
            xs = sp.tile([P, W], F32)       # pred at assigned class channel
            nc.vector.scalar_tensor_tensor(
                out=xs[:], in0=d01[:], scalar=cls_own, in1=x0,
                op0=ALU.mult, op1=ALU.add)

            # ---------- activations (Exp/Ln share ONE act table) ----------
            # sp(x) = ln(exp(x)+1);  sig(x)^2 = exp(2*(x - sp(x)))  (exact)
            es, sps, s2s_d, ts_d = [], [], [], []
            for i in range(NCHUNK):
                e = sp.tile([128, CH], F32, tag=f"d_e{i}")
                nc.scalar.activation(e[:], chs[i][:], ACT.Exp)
                es.append(e)
                spd = sp.tile([128, CH], F32, tag=f"d_sp{i}")
                nc.scalar.activation(spd[:], e[:], ACT.Ln, bias=1.0)
                sps.append(spd)
                td = sp.tile([128, CH], F32, tag=f"d_t{i}")
                nc.vector.tensor_tensor(td[:], chs[i][:], spd[:], ALU.subtract)
                ts_d.append(td)
                s2d = sp.tile([128, CH], F32, tag=f"d_s2{i}")
                nc.scalar.activation(s2d[:], td[:], ACT.Exp, scale=2.0)
                s2s_d.append(s2d)
            e_s = sp.tile([P, W], F32)
            nc.scalar.activation(e_s[:], xs[:], ACT.Exp)
            sp_s = sp.tile([P, W], F32)     # softplus at candidates
            nc.scalar.activation(sp_s[:], e_s[:], ACT.Ln, bias=1.0)
            t_s = sp.tile([P, W], F32)
            nc.vector.tensor_tensor(t_s[:], xs[:], sp_s[:], ALU.subtract)
            s2_s = sp.tile([P, W], F32)     # sigmoid^2 at candidates
            nc.scalar.activation(s2_s[:], t_s[:], ACT.Exp, scale=2.0)

            # ---------- coverage matrix [P,W,NBR] (reference predicates) ----------
            a3 = a[:, :, None].to_broadcast([P, W, NBR])
            l3 = l_n[:, None, :].to_broadcast([P, W, NBR])
            r3 = r_n[:, None, :].to_broadcast([P, W, NBR])
            rc3 = rc_n[:, None, :].to_broadcast([P, W, NBR])
            c53 = c5[:, None, :].to_broadcast([P, W, NBR])

            def t3(name):
                t = sp.tile([P, W * NBR], F32, tag=name)
                return t, t[:].rearrange("p (w m) -> p w m", m=NBR)

            # size-range sub-chain
            ls_t, ls3 = t3("b_ls")          # a - l'
            nc.vector.tensor_tensor(ls3, a3, l3, ALU.subtract)
            rs_t, rs3 = t3("b_rs")          # r' - a
            nc.vector.tensor_tensor(rs3, r3, a3, ALU.subtract)
            mx_t, mx3 = t3("b_mx")          # max(a-l', r'-a)
            nc.vector.tensor_tensor(mx3, ls3, rs3, ALU.max)
            b3_t, b33 = t3("b_b3")
            nc.vector.tensor_scalar(b33, mx3, lo, None, ALU.is_ge)
            b4_t, b43 = t3("b_b4")
            nc.vector.tensor_scalar(b43, mx3, hi, None, ALU.is_le)
            nc.vector.tensor_tensor(b33, b33, b43, ALU.mult)
            # in-box sub-chain on DVE
            b1_t, b13 = t3("b_b1")
            nc.vector.tensor_tensor(b13, a3, l3, ALU.is_ge)
            b2_t, b23 = t3("b_b2")
            nc.vector.tensor_tensor(b23, a3, rc3, ALU.is_le)
            nc.vector.tensor_tensor(b13, b13, b23, ALU.mult)
            cov_t, cov3 = t3("b_cov")       # valid_{m'}(a) for all neighbors
            nc.vector.tensor_tensor(cov3, b13, b33, ALU.mult)
            beat_t, beat3 = t3("b_beat")    # covered by strictly smaller area'
            nc.vector.tensor_tensor(beat3, cov3, c53, ALU.mult)
            btn = sp.tile([P, W], F32)
            nc.vector.reduce_max(btn[:, :, None], beat3, axis=AX.X)

            cov_self = cov_t[:].rearrange("p (w m) -> p w m", m=NBR)[:, :, K_NBR]
            pos = sp.tile([P, W], F32)      # own-valid & not beaten
            nc.vector.tensor_scalar(btn[:], btn[:], -1.0, 1.0, ALU.mult, ALU.add)
            nc.vector.tensor_tensor(pos[:], cov_self, btn[:], ALU.mult)

            # ---------- correction: pos * (sp(x) - x - 0.75*sig(x)^2*sp(x)) ----
            # contrib = sp*(1 - 0.75*sig^2) - x
            u = sp.tile([P, W], F32)
            nc.vector.tensor_scalar(u[:], s2_s[:], -0.75, 1.0, ALU.mult, ALU.add)
            nc.vector.tensor_tensor(u[:], sp_s[:], u[:], ALU.mult)
            nc.vector.tensor_tensor(u[:], u[:], xs[:], ALU.subtract)
            cn = sp.tile([128, 2], F32)     # [corr_true | npos], zero-padded to 128
            nc.vector.memset(cn[:], 0.0)
            sdump = sp.tile([P, W], F32)
            nc.vector.scalar_tensor_tensor(
                out=sdump[:], in0=u[:], scalar=0.0, in1=pos[:],
                op0=ALU.add, op1=ALU.mult, accum_out=cn[0:P, 0:1])
            nc.vector.scalar_tensor_tensor(
                out=sdump[:], in0=pos[:], scalar=0.0, in1=pos[:],
                op0=ALU.add, op1=ALU.mult, accum_out=cn[0:P, 1:2])

            # dense 0.75*sig^2*sp + row-sum, fused
            accs = sp.tile([128, NCHUNK], F32)
            dump = sp.tile([128, CH], F32, tag="d_dump")
            for i in range(NCHUNK):
                nc.vector.scalar_tensor_tensor(
                    out=dump[:], in0=s2s_d[i][:], scalar=0.75, in1=sps[i][:],
                    op0=ALU.mult, op1=ALU.mult, accum_out=accs[:, i:i + 1])

            # v[:,0] = dense_true + corr_true ; v[:,1] = npos
            v = sp.tile([128, 2], F32)
            dsum = sp.tile([128, 1], F32)
            nc.vector.reduce_sum(dsum[:], accs[:], axis=AX.X)
            nc.vector.tensor_tensor(v[0:128, 0:1], dsum[:], cn[0:128, 0:1], ALU.add)
            nc.vector.tensor_copy(v[0:128, 1:2], cn[0:128, 1:2])

            # single PE reduction: out[1,2] = ones^T @ v
            ones = sp.tile([128, 1], F32)
            nc.vector.memset(ones[:], 1.0)
            pd = ps.tile([1, 2], F32, tag="p_d")
            nc.tensor.matmul(out=pd[:], lhsT=ones[:], rhs=v[:], start=True, stop=True)
            outsb = sp.tile([1, 2], F32)
            nc.vector.tensor_copy(outsb[:], pd[:])
            nc.gpsimd.dma_start(out=out[:], in_=outsb[:])

    nc.finalize()
    return nc


_PROG = None


def _get_program():
    global _PROG
    if _PROG is None:
        _PROG = _build_program()
    return _PROG


def _prep_in_maps(pred, annotations):
    pred = np.ascontiguousarray(pred, dtype=np.float32)
    ann = np.ascontiguousarray(annotations, dtype=np.float32)

    # level constants, shared across cores
    lvlc = np.zeros((P, 8), dtype=np.float32)
    wstr = np.zeros((P, W), dtype=np.float32)
    for lvl in range(NLVL):
        s = np.float32(2.0 ** (lvl + 1))
        sl = slice(lvl * MSH, (lvl + 1) * MSH)
        lvlc[sl, 0] = s
        lvlc[sl, 1] = np.float32(2.0 ** lvl)
        lvlc[sl, 2] = SIZES[lvl, 0] * RATE
        lvlc[sl, 3] = SIZES[lvl, 1] * RATE
        lvlc[sl, 4] = np.float32(LEVEL_BASE[lvl])
        lvlc[sl, 5] = np.float32(LEVEL_BASE[lvl] + LEVEL_LENS[lvl] - 1)
        lvlc[sl, 6] = np.float32(1.0 / s)
        wstr[sl, :] = (np.arange(W, dtype=np.float32) - 1.0) * s

    # sentinel-padded annotation table for neighbor windows
    SENT = np.float32(1.0e9)
    ann_pad = np.full((M_ANN + 2 * K_NBR, 3), SENT, dtype=np.float32)
    ann_pad[:, 2] = 0.0
    ann_pad[K_NBR:K_NBR + M_ANN] = ann

    in_maps = []
    for k in range(N_CORES):
        nbr = np.zeros((MSH, 3, NBR), dtype=np.float32)
        for i in range(MSH):
            m = k * MSH + i
            nbr[i] = ann_pad[m:m + NBR].T
        ann_nbr = np.tile(nbr.reshape(MSH, 3 * NBR), (NLVL, 1))  # [80, 27]
        aux = np.concatenate([ann_nbr, lvlc, wstr], axis=1)      # [80, 43]
        in_maps.append({
            "pred_full": pred,
            "pred_slice": np.ascontiguousarray(pred[k * NSH:(k + 1) * NSH]),
            "aux": np.ascontiguousarray(aux),
        })
    return in_maps


def _finalize(outs):
    num = np.sum([o[0, 0] for o in outs], dtype=np.float64)
    npos = np.sum([o[0, 1] for o in outs], dtype=np.float64)
    return np.float32(num / max(npos, 1.0))


def kernel(pred, annotations, anchors0=None, anchors1=None, anchors2=None,
           anchors3=None, anchors4=None, **_ignored):
    nc = _get_program()
    in_maps = _prep_in_maps(np.asarray(pred), np.asarray(annotations))

    if os.environ.get("KERNEL_SIM") == "1":
        from concourse import bass_interp
        outs = []
        for k in range(N_CORES):
            sim = bass_interp.CoreSim(nc)
            for name, val in in_maps[k].items():
                sim.tensor(name)[:] = val
            sim.simulate()
            outs.append(np.array(sim.tensor("out")))
        return _finalize(outs)

    from concourse import bass_utils
    res = bass_utils.run_bass_kernel_spmd(nc, in_maps, core_ids=list(range(N_CORES)))
    return _finalize([r["out"] for r in res.results])
